# revision 17
# baseline (speedup 1.0000x reference)
"""Trainium2 Bass kernel for nn_Event_Critic_Net (dual-branch GAT critic).

Math: the reference reads the GAT output only at the LAST node of each
graph (graphs are 32 contiguous nodes), so only edges whose dst is a
graph's last node contribute:

    out_g = sigmoid( (sum_n alpha[n] * x[n,:]) @ W + bias )
    alpha[n] = cnt[n]*exp(e[n]) / (sum_n cnt[n]*exp(e[n]) + 1e-16)
    e[n] = leaky_relu(x[n]. w_src + x[last(g)]. w_dst),  w_* = W @ att_*

cnt[n] = #edges (n -> last(g)).  Nodes with cnt==0 are dropped on the
host; survivors (~7/graph) are packed whole-graph into 128-slot tiles
via first-fit-decreasing (T=32 tiles/core/branch, measured need <=30).

Device pipeline per branch (u then d, pipelined across engines):
  1. advb MMs: a_dst per gd slot (xlT chunks x wd2) -> PSUM.
  2. a_src MMs: 16 two-block xt chunks x wv2 -> e PSUM (start, no stop)
  3. advs copy (ACT), advm = advs*Bm4 (DVE), 8 mkT scatter MMs
     accumulate a_dst INTO the same e PSUM cols (stop=True).
  4. leaky (DVE from PSUM), exp (ACT), P = cnt*exp (DVE).
  5. mkP = mk (*) P : mask columns scaled by P (DVE); aggregation MMs
     use mkP stationary x raw xab tiles -> yps (ones col -> denom).
  6. normalize (DVE from PSUM), 8 PE transposes -> ynT, project (2 MMs),
     sigmoid (ACT; table-set load prefetched right after the last exp
     via a dummy 1-elem sigmoid), prod (DVE), mlp (2 MMs), bias, DMA.

DMA: 3 queues (sync/scalar HWDGE, gpsimd SWDGE); masks shipped as
fp8_e4m3 and cast to bf16 during the gpsimd DMA (halves their bytes).
Host un-permutes via the packing's graph->gd map (gd = out column).
"""

import numpy as np
from contextlib import ExitStack

NC = 8            # cores
N = 131072        # nodes total
G = 4096          # graphs
NPG = 32          # nodes per graph
S = 64            # state size
H = 128           # hidden size
NPC = N // NC     # 16384 nodes per core
GPC = G // NC     # 512 graphs per core
SA = 66           # x columns: 64 features | ones | zero pad
T = 30            # packed node tiles per core per branch (measured max)
EP = 32           # padded e-region stride (scatter block 7 spills to col 31)
NB = 8            # mkT blocks / gd blocks of 128 (covers 32 tile slots)
NCH = T // 2      # 15 two-block xt chunks
GD = NB * 128     # 1024 gd slots
NEG = 0.2

# cbF (bf16, front consts) column layout
CB_V2U = 0                     # wv2_u [128,2] (2-block ws)
CB_V2D = CB_V2U + 2
CB_D2U = CB_V2D + 2            # wd2_u [64,1]
CB_D2D = CB_D2U + 1
CB_BM4 = CB_D2D + 1            # Bm4 [128,4]
CBFW = CB_BM4 + 4
# cbL (bf16, late consts) column layout
CB_ID = 0                      # identity [128,128]
CB_WSU = CB_ID + 128           # Ws_u rows 0:64 [*,128]
CB_WSD = CB_WSU + 128
CB_MLP = CB_WSD + 128          # 0.25*mlpW [128,1]
CBLW = CB_MLP + 1

# cf (f32) column layout
CF_EPS = 0                     # 1e-16
CF_MLB = 1                     # mlp_b
CF_BU = 2                      # bias_u per-partition
CF_BD = 3
CFW = 4

_CACHE = {}


def _build_module():
    import concourse.tile as tile
    from concourse import bacc, mybir
    from concourse.alu_op_type import AluOpType as Alu

    f32 = mybir.dt.float32
    bf16 = mybir.dt.bfloat16
    f8 = mybir.dt.float8e4
    Act = mybir.ActivationFunctionType

    nc = bacc.Bacc("TRN2", target_bir_lowering=False, debug=False,
                   num_devices=NC)

    dram = {}

    def din(name, shape, dt=bf16):
        dram[name] = nc.dram_tensor(name, shape, dt, kind="ExternalInput")

    for p in ("u", "d"):
        din(f"{p}_xab", [128, T * SA])
        din(f"{p}_xt", [128, NCH * 128 + (CBFW if p == "u" else 0)])
        din(f"{p}_xlT", [64, GD])
        din(f"{p}_mk", [128, T * 32], f8)
        din(f"{p}_mkT", [128, NB * 128], f8)
    din("cbL", [128, CBLW])
    din("cf", [128, CFW], f32)
    out_dram = nc.dram_tensor("out", [1, GD], f32, kind="ExternalOutput")

    with tile.TileContext(nc) as tc, ExitStack() as ctx:
        const = ctx.enter_context(tc.tile_pool(name="const", bufs=1))
        xp = ctx.enter_context(tc.tile_pool(name="xp", bufs=1))
        wk = ctx.enter_context(tc.tile_pool(name="wk", bufs=1))
        ps1 = ctx.enter_context(tc.tile_pool(name="ps1", bufs=1, space="PSUM"))

        cbL = const.tile([128, CBLW], bf16, tag="cbL")
        cf = const.tile([128, CFW], f32, tag="cf")
        st = {"u": {}, "d": {}}
        for p in ("u", "d"):
            s = st[p]
            s["xab"] = xp.tile([128, T * SA], bf16, tag=f"xab_{p}",
                               name=f"xab_{p}")
            s["xt"] = xp.tile([128, NCH * 128 + (CBFW if p == "u" else 0)],
                              bf16, tag=f"xt_{p}", name=f"xt_{p}")
            s["xlT"] = xp.tile([64, GD], bf16, tag=f"xlT_{p}",
                               name=f"xlT_{p}")
            s["mk"] = xp.tile([128, T * 32], bf16, tag=f"mk_{p}",
                              name=f"mk_{p}")
            s["mkT"] = xp.tile([128, NB * 128], bf16, tag=f"mkT_{p}",
                               name=f"mkT_{p}")

        # ---- DMA enqueues (3 queues; front tensors first) ----
        nc.scalar.dma_start(st["u"]["xlT"][:], dram["u_xlT"].ap())
        UXT1 = 8 * 128
        nc.scalar.dma_start(st["u"]["xt"][:, 0:UXT1],
                            dram["u_xt"].ap()[:, 0:UXT1])
        nc.scalar.dma_start(st["u"]["xt"][:, UXT1:],
                            dram["u_xt"].ap()[:, UXT1:])
        nc.sync.dma_start(st["u"]["xab"][:], dram["u_xab"].ap())
        nc.sync.dma_start(st["d"]["xt"][:], dram["d_xt"].ap())
        nc.sync.dma_start(cbL[:], dram["cbL"].ap())
        nc.sync.dma_start(cf[:], dram["cf"].ap())
        nc.gpsimd.dma_start(st["d"]["xlT"][:], dram["d_xlT"].ap())
        nc.gpsimd.dma_start(st["u"]["mkT"][:], dram["u_mkT"].ap())
        nc.gpsimd.dma_start(st["u"]["mk"][:], dram["u_mk"].ap())
        nc.gpsimd.dma_start(st["d"]["mkT"][:], dram["d_mkT"].ap())
        nc.gpsimd.dma_start(st["d"]["mk"][:], dram["d_mk"].ap())
        nc.gpsimd.dma_start(st["d"]["xab"][:], dram["d_xab"].ap())

        identb = cbL[:, CB_ID:CB_ID + 128]
        Ws = {"u": cbL[0:S, CB_WSU:CB_WSU + 128],
              "d": cbL[0:S, CB_WSD:CB_WSD + 128]}
        cbF = st["u"]["xt"][:, NCH * 128:NCH * 128 + CBFW]
        wv2 = {"u": cbF[:, CB_V2U:CB_V2U + 2],
               "d": cbF[:, CB_V2D:CB_V2D + 2]}
        wd2 = {"u": cbF[0:S, CB_D2U:CB_D2U + 1],
               "d": cbF[0:S, CB_D2D:CB_D2D + 1]}
        mlpW = cbL[:, CB_MLP:CB_MLP + 1]
        Bm4 = cbF[:, CB_BM4:CB_BM4 + 4]  # slice of u_xt tile
        eps = cf[:, CF_EPS:CF_EPS + 1]
        mlpb = cf[0:1, CF_MLB:CF_MLB + 1]
        biases = {"u": cf[:, CF_BU:CF_BU + 1], "d": cf[:, CF_BD:CF_BD + 1]}

        # ---- PSUM tiles (8 banks exactly) ----
        # epsud: e_u | e_d | advb_u | advb_d
        epsud = ps1.tile([128, 4 * EP], f32, tag="epsud")
        # per-branch y PSUM: blocks 0..6 at col 66*b, block 7 at col 512
        yps = {"u": ps1.tile([128, 578 + NB], f32, tag="yps_u", name="yps_u"),
               "d": ps1.tile([128, 578 + NB], f32, tag="yps_d", name="yps_d")}
        ytp = ps1.tile([64, NB * 128], bf16, tag="ytp")      # shared u->d
        hT = ps1.tile([128, 1024], f32, tag="hT")            # proj + mlp

        def ycol(b):
            return SA * b if b < 7 else 512

        # ---- SBUF work tiles ----
        advm = {p: wk.tile([128, NB * 4], bf16, tag=f"advm_{p}",
                           name=f"advm_{p}") for p in "ud"}
        ee = {p: wk.tile([128, T], f32, tag=f"ee_{p}", name=f"ee_{p}")
              for p in "ud"}
        es = {p: wk.tile([128, T], f32, tag=f"es_{p}", name=f"es_{p}")
              for p in "ud"}
        adsb = {p: wk.tile([128, T], f32, tag=f"adsb_{p}", name=f"adsb_{p}")
                for p in "ud"}
        ex = {p: wk.tile([128, T], f32, tag=f"ex_{p}", name=f"ex_{p}")
              for p in "ud"}
        mkP = {p: wk.tile([128, T * 32], bf16, tag=f"mkP_{p}",
                          name=f"mkP_{p}") for p in "ud"}
        dn = {p: wk.tile([128, NB], f32, tag=f"dn_{p}", name=f"dn_{p}")
              for p in "ud"}
        rp = {p: wk.tile([128, NB], f32, tag=f"rp_{p}", name=f"rp_{p}")
              for p in "ud"}
        ynrm = {p: wk.tile([128, NB * S], bf16, tag=f"ynrm_{p}",
                           name=f"ynrm_{p}") for p in "ud"}
        ynT = {p: wk.tile([64, GD], bf16, tag=f"ynT_{p}", name=f"ynT_{p}")
               for p in "ud"}
        sg = {p: wk.tile([128, GD], bf16, tag=f"sg_{p}", name=f"sg_{p}")
              for p in "ud"}
        prod = wk.tile([128, GD], bf16, tag="prod")
        o_sb = wk.tile([1, GD], f32, tag="o_sb")

        def front(p, po):
            s = st[p]
            # a_dst per gd: 8 xlT-chunk MMs -> advb cols
            for b in range(NB):
                nc.tensor.matmul(
                    yps[p][:, 578 + b:578 + b + 1],
                    s["xlT"][:, 128 * b:128 * (b + 1)],
                    wd2[p], start=True, stop=True)
            # advm[:, b, n] = advb[:, b] * Bm4[:, n]  (one fused stt)
            nc.vector.scalar_tensor_tensor(
                advm[p][:].rearrange("q (b n) -> q b n", n=4),
                yps[p][:, 578:578 + NB][:, :, None].broadcast_to(
                    (128, NB, 4)),
                1.0,
                Bm4[:, None, :].broadcast_to((128, NB, 4)),
                op0=Alu.mult, op1=Alu.mult)
            # scatter a_dst into the e cols (accumulate, close group)
            for b in range(NB):
                nc.tensor.matmul(
                    epsud[:, 2 * EP + EP * po + 4 * b:2 * EP + EP * po + 4 * b + 4],
                    s["mkT"][:, 128 * b:128 * (b + 1)],
                    advm[p][:, 4 * b:4 * b + 4],
                    start=True, stop=True)
            # a_src chunk MMs
            for c in range(NCH):
                nc.tensor.matmul(
                    epsud[:, EP * po + 2 * c:EP * po + 2 * c + 2],
                    s["xt"][:, 128 * c:128 * (c + 1)],
                    wv2[p], start=True, stop=True)
            # e = e_src + e_dst; P = cnt * exp(leaky(e))
            nc.vector.tensor_scalar(
                adsb[p][:], epsud[:, 2 * EP + EP * po:2 * EP + EP * po + T],
                0.0, None, op0=Alu.add)
            nc.vector.scalar_tensor_tensor(
                es[p][:], epsud[:, EP * po:EP * po + T], 1.0, adsb[p][:],
                op0=Alu.mult, op1=Alu.add)
            nc.vector.scalar_tensor_tensor(
                ee[p][:], es[p][:], NEG, es[p][:], op0=Alu.mult, op1=Alu.max)
            nc.scalar.activation(ex[p][:], ee[p][:], Act.Exp)
            # mkP = (mk*cnt) (*) exp(e)   (cnt pre-folded on host)
            nc.vector.tensor_tensor(
                mkP[p][:].rearrange("q (t j) -> q t j", j=32),
                s["mk"][:].rearrange("q (t j) -> q t j", j=32),
                ex[p][:, :, None].broadcast_to((128, T, 32)),
                op=Alu.mult)

        def agg(p):
            s = st[p]
            yb = yps[p]
            for t in range(T):
                nc.tensor.matmul(
                    yb[32 * (t % 4):32 * (t % 4) + 32,
                       ycol(t // 4):ycol(t // 4) + SA],
                    mkP[p][:, 32 * t:32 * (t + 1)],
                    s["xab"][:, SA * t:SA * (t + 1)],
                    start=True, stop=True,
                    tile_position=(0, 32 * (t % 4)))

        def norm_proj(p):
            yb = yps[p]
            nc.vector.tensor_scalar(
                dn[p][:, 0:7],
                yb[:, 0:7 * SA].rearrange("q (b f) -> q b f", f=SA)[:, :, S],
                eps, None, op0=Alu.add)
            nc.vector.tensor_scalar(
                dn[p][:, 7:8], yb[:, 512 + S:512 + S + 1], eps, None,
                op0=Alu.add)
            nc.vector.reciprocal_approx_fast(rp[p][:], dn[p][:])
            nc.vector.tensor_tensor(
                ynrm[p][:].rearrange("q (b f) -> q b f", f=S)[:, 0:7],
                yb[:, 0:7 * SA].rearrange("q (b f) -> q b f", f=SA)[:, :, 0:S],
                rp[p][:, 0:7, None].broadcast_to((128, 7, S)),
                op=Alu.mult)
            nc.vector.tensor_tensor(
                ynrm[p][:, 7 * S:8 * S],
                yb[:, 512:512 + S],
                rp[p][:, 7:8].broadcast_to((128, S)),
                op=Alu.mult)
            for b in range(NB):
                nc.tensor.transpose(
                    ytp[:, 128 * b:128 * (b + 1)],
                    ynrm[p][:, S * b:S * (b + 1)],
                    identb, tile_position=(0, 0))
            nc.vector.tensor_copy(ynT[p][:], ytp[:])
            for i in range(2):
                nc.tensor.matmul(hT[:, 512 * i:512 * (i + 1)], Ws[p],
                                 ynT[p][:, 512 * i:512 * (i + 1)],
                                 start=True, stop=True)
            nc.scalar.activation(sg[p][:], hT[:], Act.Tanh,
                                 bias=biases[p], scale=0.5)

        front("u", 0)
        front("d", 1)
        agg("u")
        agg("d")
        norm_proj("u")
        norm_proj("d")

        nc.vector.tensor_tensor(prod[:], sg["u"][:], sg["d"][:], op=Alu.mult)
        for i in range(2):
            for k, rhs in enumerate((prod, sg["u"], sg["d"])):
                nc.tensor.matmul(hT[0:1, 512 * i:512 * (i + 1)], mlpW,
                                 rhs[:, 512 * i:512 * (i + 1)],
                                 start=(k == 0), stop=(k == 2))
            nc.vector.tensor_scalar(
                o_sb[:, 512 * i:512 * (i + 1)],
                hT[0:1, 512 * i:512 * (i + 1)], mlpb, None, op0=Alu.add)
        nc.sync.dma_start(out_dram.ap(), o_sb[:])

    nc.compile()
    return nc


def _get_module():
    if "nc" not in _CACHE:
        _CACHE["nc"] = _build_module()
    return _CACHE["nc"]


def _nz_counts(ei):
    src = np.asarray(ei[0]).astype(np.int64)
    dst = np.asarray(ei[1]).astype(np.int64)
    valid = (dst % NPG) == (NPG - 1)
    return np.bincount(src[valid], minlength=N).astype(np.float32)


def _pack_core(cnt_u, cnt_d, base):
    """First-fit-decreasing whole-graph packing for one core (shared
    graph->tile map for both branches)."""
    ku = [np.nonzero(cnt_u[base + NPG * g: base + NPG * (g + 1)])[0]
          for g in range(GPC)]
    kd = [np.nonzero(cnt_d[base + NPG * g: base + NPG * (g + 1)])[0]
          for g in range(GPC)]
    nu = np.array([len(k) for k in ku])
    nd = np.array([len(k) for k in kd])
    order = np.argsort(-(nu + nd), kind="stable")
    bins = []
    for g in order:
        placed = False
        for b in bins:
            if b[0] + nu[g] <= 128 and b[1] + nd[g] <= 128 and len(b[2]) < 32:
                b[0] += nu[g]
                b[1] += nd[g]
                b[2].append(g)
                placed = True
                break
        if not placed:
            bins.append([nu[g], nd[g], [g]])
    assert len(bins) <= T, f"packing needs {len(bins)} tiles > {T}"
    return [b[2] for b in bins], ku, kd


def _branch_arrays(tiles, klists, cnt, x, base):
    import ml_dtypes
    bf = ml_dtypes.bfloat16
    f8 = ml_dtypes.float8_e4m3
    SLOTS = T * 128
    xs = np.zeros((SLOTS, S), np.float32)
    cs = np.zeros(SLOTS, np.float32)
    mk = np.zeros((T, 128, 32), np.float32)
    mkT = np.zeros((128, NB * 128), np.float32)
    for t, gs in enumerate(tiles):
        off = 0
        for j, g in enumerate(gs):
            nodes = base + NPG * g + klists[g]
            k = len(nodes)
            xs[128 * t + off:128 * t + off + k] = x[nodes]
            cs[128 * t + off:128 * t + off + k] = cnt[nodes]
            mk[t, off:off + k, j] = cnt[nodes]
            mkT[32 * (t % 4) + j, 128 * (t // 4) + off:
                128 * (t // 4) + off + k] = 1.0
            off += k

    xab = np.zeros((T, 128, SA), np.float32)
    xab[:, :, :S] = xs.reshape(T, 128, S)
    xab[:, :, S] = 1.0
    xab = np.ascontiguousarray(
        xab.transpose(1, 0, 2).reshape(128, T * SA)).astype(bf)
    # xt: 2-block chunks — chunk c = even tile 2c (rows 0:64) and odd
    # tile 2c+1 (rows 64:128), feat-major
    xst = xs.reshape(T, 128, S)
    xtv = np.concatenate([
        xst[0::2].reshape(NCH * 128, S).T,
        xst[1::2].reshape(NCH * 128, S).T], axis=0)
    xtv = np.ascontiguousarray(xtv).astype(bf)
    cnt_t = np.ascontiguousarray(cs.reshape(T, 128).T)
    mk2 = np.ascontiguousarray(
        mk.transpose(1, 0, 2).reshape(128, T * 32)).astype(f8)
    return {"xab": xab, "xt": xtv, "mk": mk2, "mkT": mkT.astype(f8),
            "cnt": cnt_t}


def _build_in_maps(inputs):
    import ml_dtypes
    bf = ml_dtypes.bfloat16
    data = {}
    for p, pref in (("u", "up"), ("d", "down")):
        W = np.asarray(inputs[f"{pref}_W"], np.float32)
        data[p] = {
            "x": np.asarray(inputs[f"{pref}_x"], np.float32),
            "cnt": _nz_counts(inputs[f"{pref}_edge_index"]),
            "ws": W @ np.asarray(inputs[f"{pref}_att_src"], np.float32),
            "wd": W @ np.asarray(inputs[f"{pref}_att_dst"], np.float32),
            "W": W,
            "bias": np.asarray(inputs[f"{pref}_bias"], np.float32).reshape(H),
        }

    cbf = np.zeros((128, CBFW), np.float32)
    cbf[0:S, CB_V2U] = data["u"]["ws"]
    cbf[S:, CB_V2U + 1] = data["u"]["ws"]
    cbf[0:S, CB_V2D] = data["d"]["ws"]
    cbf[S:, CB_V2D + 1] = data["d"]["ws"]
    cbf[0:S, CB_D2U] = data["u"]["wd"]
    cbf[0:S, CB_D2D] = data["d"]["wd"]
    cbf[np.arange(128), CB_BM4 + np.arange(128) // 32] = 1.0
    cbl = np.zeros((128, CBLW), np.float32)
    cbl[:, CB_ID:CB_ID + 128] = np.eye(128)
    cbl[0:S, CB_WSU:CB_WSU + 128] = data["u"]["W"]
    cbl[0:S, CB_WSD:CB_WSD + 128] = data["d"]["W"]
    mw = np.asarray(inputs["mlp_W"], np.float32).reshape(H)
    cbl[:, CB_MLP] = 0.25 * mw
    cbf = cbf.astype(bf)
    cbl = cbl.astype(bf)

    cf = np.zeros((128, CFW), np.float32)
    cf[:, CF_EPS] = 1e-16
    mw = np.asarray(inputs["mlp_W"], np.float32).reshape(H)
    cf[0, CF_MLB] = float(np.asarray(inputs["mlp_b"]).reshape(-1)[0]) \
        + 0.25 * float(mw.sum())
    cf[:, CF_BU] = 0.5 * data["u"]["bias"]
    cf[:, CF_BD] = 0.5 * data["d"]["bias"]

    in_maps = []
    perms = []
    for c in range(NC):
        base = c * NPC
        tiles, ku, kd = _pack_core(data["u"]["cnt"], data["d"]["cnt"], base)
        gd_of_g = np.full(GPC, -1, np.int64)
        for t, gs in enumerate(tiles):
            for j, g in enumerate(gs):
                gd_of_g[g] = 32 * t + j
        assert (gd_of_g >= 0).all()
        perms.append(gd_of_g)
        m = {"cbL": cbl}
        cfc = cf.copy()
        for p, kl in (("u", ku), ("d", kd)):
            arrs = _branch_arrays(tiles, kl, data[p]["cnt"],
                                  data[p]["x"], base)
            for kk in ("xab", "xt", "mk", "mkT"):
                m[f"{p}_{kk}"] = arrs[kk]
            if p == "u":
                m["u_xt"] = np.concatenate([m["u_xt"], cbf], axis=1)
            # x_last feat-major by gd slot
            xlT = np.zeros((S, GD), np.float32)
            gl = np.arange(GPC)
            last = data[p]["x"][base + NPG * gl + NPG - 1]   # [GPC, S]
            xlT[:, gd_of_g] = last.T
            m[f"{p}_xlT"] = xlT.astype(bf)
        m["cf"] = cfc
        in_maps.append(m)
    return in_maps, perms


def kernel(**inputs):
    from concourse.bass_utils import run_bass_kernel_spmd

    nc = _get_module()
    in_maps, perms = _build_in_maps(inputs)
    res = run_bass_kernel_spmd(nc, in_maps, core_ids=list(range(NC)))
    out = np.empty((NC, GPC), np.float32)
    for c, r in enumerate(res.results):
        full = np.asarray(r["out"], np.float32).reshape(GD)
        out[c] = full[perms[c]]
    return out.reshape(G, 1)


# revision 18
# speedup vs baseline: 1.0525x; 1.0525x over previous
"""Trainium2 Bass kernel for nn_Event_Critic_Net (dual-branch GAT critic).

Math: the reference reads the GAT output only at the LAST node of each
graph (graphs are 32 contiguous nodes), so only edges whose dst is a
graph's last node contribute:

    out_g = sigmoid( (sum_n alpha[n] * x[n,:]) @ W + bias )
    alpha[n] = cnt[n]*exp(e[n]) / (sum_n cnt[n]*exp(e[n]) + 1e-16)
    e[n] = leaky_relu(x[n]. w_src + x[last(g)]. w_dst),  w_* = W @ att_*

cnt[n] = #edges (n -> last(g)).  Nodes with cnt==0 are dropped on the
host; survivors (~7/graph) are packed whole-graph into 128-slot tiles
via first-fit-decreasing (T=32 tiles/core/branch, measured need <=30).

Device pipeline per branch (u then d, pipelined across engines):
  1. advb MMs: a_dst per gd slot (xlT chunks x wd2) -> PSUM.
  2. a_src MMs: 16 two-block xt chunks x wv2 -> e PSUM (start, no stop)
  3. advs copy (ACT), advm = advs*Bm4 (DVE), 8 mkT scatter MMs
     accumulate a_dst INTO the same e PSUM cols (stop=True).
  4. leaky (DVE from PSUM), exp (ACT), P = cnt*exp (DVE).
  5. mkP = mk (*) P : mask columns scaled by P (DVE); aggregation MMs
     use mkP stationary x raw xab tiles -> yps (ones col -> denom).
  6. normalize (DVE from PSUM), 8 PE transposes -> ynT, project (2 MMs),
     sigmoid (ACT; table-set load prefetched right after the last exp
     via a dummy 1-elem sigmoid), prod (DVE), mlp (2 MMs), bias, DMA.

DMA: 3 queues (sync/scalar HWDGE, gpsimd SWDGE); masks shipped as
fp8_e4m3 and cast to bf16 during the gpsimd DMA (halves their bytes).
Host un-permutes via the packing's graph->gd map (gd = out column).
"""

import numpy as np
from contextlib import ExitStack

NC = 8            # cores
N = 131072        # nodes total
G = 4096          # graphs
NPG = 32          # nodes per graph
S = 64            # state size
H = 128           # hidden size
NPC = N // NC     # 16384 nodes per core
GPC = G // NC     # 512 graphs per core
SA = 66           # x columns: 64 features | ones | zero pad
T = 30            # packed node tiles per core per branch (measured max)
EP = 32           # padded e-region stride (scatter block 7 spills to col 31)
NB = 8            # mkT blocks / gd blocks of 128 (covers 32 tile slots)
NCH = T // 2      # 15 two-block xt chunks
GD = NB * 128     # 1024 gd slots
NEG = 0.2

# cbF (bf16, front consts) column layout
CB_V2U = 0                     # wv2_u [128,2] (2-block ws)
CB_V2D = CB_V2U + 2
CB_D2U = CB_V2D + 2            # wd2_u [64,1]
CB_D2D = CB_D2U + 1
CB_BM4 = CB_D2D + 1            # Bm4 [128,4]
CBFW = CB_BM4 + 4
# cbL (bf16, late consts) column layout
CB_ID = 0                      # identity [128,128]
CB_WSU = CB_ID + 128           # Ws_u rows 0:64 [*,128]
CB_WSD = CB_WSU + 128
CB_MLP = CB_WSD + 128          # 0.25*mlpW [128,1]
CBLW = CB_MLP + 1

# cf (f32) column layout
CF_EPS = 0                     # 1e-16
CF_MLB = 1                     # mlp_b
CF_BU = 2                      # bias_u per-partition
CF_BD = 3
CFW = 4

_CACHE = {}


def _build_module():
    import concourse.tile as tile
    from concourse import bacc, mybir
    from concourse.alu_op_type import AluOpType as Alu

    f32 = mybir.dt.float32
    bf16 = mybir.dt.bfloat16
    f8 = mybir.dt.float8e4
    Act = mybir.ActivationFunctionType

    nc = bacc.Bacc("TRN2", target_bir_lowering=False, debug=False,
                   num_devices=NC)

    dram = {}

    def din(name, shape, dt=bf16):
        dram[name] = nc.dram_tensor(name, shape, dt, kind="ExternalInput")

    for p in ("u", "d"):
        din(f"{p}_xab", [128, T * SA])
        din(f"{p}_xt", [128, NCH * 128 + (CBFW if p == "u" else 0)])
        din(f"{p}_xlT", [64, GD])
        din(f"{p}_mk", [128, T * 32], f8)
        din(f"{p}_mkT", [128, NB * 128], f8)
    din("cbL", [128, CBLW])
    din("cf", [128, CFW], f32)
    out_dram = nc.dram_tensor("out", [1, GD], f32, kind="ExternalOutput")

    with tile.TileContext(nc) as tc, ExitStack() as ctx:
        const = ctx.enter_context(tc.tile_pool(name="const", bufs=1))
        xp = ctx.enter_context(tc.tile_pool(name="xp", bufs=1))
        wk = ctx.enter_context(tc.tile_pool(name="wk", bufs=1))
        ps1 = ctx.enter_context(tc.tile_pool(name="ps1", bufs=1, space="PSUM"))

        cbL = const.tile([128, CBLW], bf16, tag="cbL")
        cf = const.tile([128, CFW], f32, tag="cf")
        st = {"u": {}, "d": {}}
        for p in ("u", "d"):
            s = st[p]
            s["xab"] = xp.tile([128, T * SA], bf16, tag=f"xab_{p}",
                               name=f"xab_{p}")
            s["xt"] = xp.tile([128, NCH * 128 + (CBFW if p == "u" else 0)],
                              bf16, tag=f"xt_{p}", name=f"xt_{p}")
            s["xlT"] = xp.tile([64, GD], bf16, tag=f"xlT_{p}",
                               name=f"xlT_{p}")
            s["mk"] = xp.tile([128, T * 32], bf16, tag=f"mk_{p}",
                              name=f"mk_{p}")
            s["mkT"] = xp.tile([128, NB * 128], bf16, tag=f"mkT_{p}",
                               name=f"mkT_{p}")

        # ---- DMA enqueues (3 queues; front tensors first) ----
        nc.scalar.dma_start(st["u"]["xlT"][:], dram["u_xlT"].ap())
        UXT1 = 8 * 128
        nc.scalar.dma_start(st["u"]["xt"][:, 0:UXT1],
                            dram["u_xt"].ap()[:, 0:UXT1])
        nc.scalar.dma_start(st["u"]["xt"][:, UXT1:],
                            dram["u_xt"].ap()[:, UXT1:])
        nc.sync.dma_start(st["d"]["xt"][:], dram["d_xt"].ap())
        nc.sync.dma_start(st["u"]["xab"][:], dram["u_xab"].ap())
        nc.sync.dma_start(cbL[:], dram["cbL"].ap())
        nc.sync.dma_start(cf[:], dram["cf"].ap())
        nc.gpsimd.dma_start(st["d"]["xlT"][:], dram["d_xlT"].ap())
        nc.gpsimd.dma_start(st["u"]["mkT"][:], dram["u_mkT"].ap())
        nc.gpsimd.dma_start(st["u"]["mk"][:], dram["u_mk"].ap())
        nc.gpsimd.dma_start(st["d"]["mkT"][:], dram["d_mkT"].ap())
        nc.gpsimd.dma_start(st["d"]["mk"][:], dram["d_mk"].ap())
        nc.gpsimd.dma_start(st["d"]["xab"][:], dram["d_xab"].ap())

        identb = cbL[:, CB_ID:CB_ID + 128]
        Ws = {"u": cbL[0:S, CB_WSU:CB_WSU + 128],
              "d": cbL[0:S, CB_WSD:CB_WSD + 128]}
        cbF = st["u"]["xt"][:, NCH * 128:NCH * 128 + CBFW]
        wv2 = {"u": cbF[:, CB_V2U:CB_V2U + 2],
               "d": cbF[:, CB_V2D:CB_V2D + 2]}
        wd2 = {"u": cbF[0:S, CB_D2U:CB_D2U + 1],
               "d": cbF[0:S, CB_D2D:CB_D2D + 1]}
        mlpW = cbL[:, CB_MLP:CB_MLP + 1]
        Bm4 = cbF[:, CB_BM4:CB_BM4 + 4]  # slice of u_xt tile
        eps = cf[:, CF_EPS:CF_EPS + 1]
        mlpb = cf[0:1, CF_MLB:CF_MLB + 1]
        biases = {"u": cf[:, CF_BU:CF_BU + 1], "d": cf[:, CF_BD:CF_BD + 1]}

        # ---- PSUM tiles (8 banks exactly) ----
        # epsud: e_u | e_d | advb_u | advb_d
        epsud = ps1.tile([128, 4 * EP], f32, tag="epsud")
        # per-branch y PSUM: blocks 0..6 at col 66*b, block 7 at col 512
        yps = {"u": ps1.tile([128, 578 + NB], f32, tag="yps_u", name="yps_u"),
               "d": ps1.tile([128, 578 + NB], f32, tag="yps_d", name="yps_d")}
        ytp = ps1.tile([64, NB * 128], bf16, tag="ytp")      # shared u->d
        hT = ps1.tile([128, 1024], f32, tag="hT")            # proj + mlp

        def ycol(b):
            return SA * b if b < 7 else 512

        # ---- SBUF work tiles ----
        advm = {p: wk.tile([128, NB * 4], bf16, tag=f"advm_{p}",
                           name=f"advm_{p}") for p in "ud"}
        ee = {p: wk.tile([128, T], f32, tag=f"ee_{p}", name=f"ee_{p}")
              for p in "ud"}
        es = {p: wk.tile([128, T], f32, tag=f"es_{p}", name=f"es_{p}")
              for p in "ud"}
        adsb = {p: wk.tile([128, T], f32, tag=f"adsb_{p}", name=f"adsb_{p}")
                for p in "ud"}
        ex = {p: wk.tile([128, T], f32, tag=f"ex_{p}", name=f"ex_{p}")
              for p in "ud"}
        mkP = {p: wk.tile([128, T * 32], bf16, tag=f"mkP_{p}",
                          name=f"mkP_{p}") for p in "ud"}
        dn = {p: wk.tile([128, NB], f32, tag=f"dn_{p}", name=f"dn_{p}")
              for p in "ud"}
        rp = {p: wk.tile([128, NB], f32, tag=f"rp_{p}", name=f"rp_{p}")
              for p in "ud"}
        ynrm = {p: wk.tile([128, NB * S], bf16, tag=f"ynrm_{p}",
                           name=f"ynrm_{p}") for p in "ud"}
        ynT = {p: wk.tile([64, GD], bf16, tag=f"ynT_{p}", name=f"ynT_{p}")
               for p in "ud"}
        sg = {p: wk.tile([128, GD], bf16, tag=f"sg_{p}", name=f"sg_{p}")
              for p in "ud"}
        prod = wk.tile([128, GD], bf16, tag="prod")
        o_sb = wk.tile([1, GD], f32, tag="o_sb")

        dumb = wk.tile([128, 512], bf16, tag="dumb")
        nc.vector.memset(dumb[:], 0.0)
        for i in range(6):
            nc.tensor.matmul(hT[:, 0:512], dumb[:, 0:128], dumb[:],
                             start=True, stop=True)

        def front(p, po):
            s = st[p]
            # a_dst per gd: 8 xlT-chunk MMs -> advb cols
            for b in range(NB):
                nc.tensor.matmul(
                    yps[p][:, 578 + b:578 + b + 1],
                    s["xlT"][:, 128 * b:128 * (b + 1)],
                    wd2[p], start=True, stop=True)
            # advm[:, b, n] = advb[:, b] * Bm4[:, n]  (one fused stt)
            nc.vector.scalar_tensor_tensor(
                advm[p][:].rearrange("q (b n) -> q b n", n=4),
                yps[p][:, 578:578 + NB][:, :, None].broadcast_to(
                    (128, NB, 4)),
                1.0,
                Bm4[:, None, :].broadcast_to((128, NB, 4)),
                op0=Alu.mult, op1=Alu.mult)
            # scatter a_dst into the e cols (accumulate, close group)
            for b in range(NB):
                nc.tensor.matmul(
                    epsud[:, 2 * EP + EP * po + 4 * b:2 * EP + EP * po + 4 * b + 4],
                    s["mkT"][:, 128 * b:128 * (b + 1)],
                    advm[p][:, 4 * b:4 * b + 4],
                    start=True, stop=True)
            # a_src chunk MMs
            for c in range(NCH):
                nc.tensor.matmul(
                    epsud[:, EP * po + 2 * c:EP * po + 2 * c + 2],
                    s["xt"][:, 128 * c:128 * (c + 1)],
                    wv2[p], start=True, stop=True)
            # e = e_src + e_dst; P = cnt * exp(leaky(e))
            nc.vector.tensor_scalar(
                adsb[p][:], epsud[:, 2 * EP + EP * po:2 * EP + EP * po + T],
                0.0, None, op0=Alu.add)
            nc.vector.scalar_tensor_tensor(
                es[p][:], epsud[:, EP * po:EP * po + T], 1.0, adsb[p][:],
                op0=Alu.mult, op1=Alu.add)
            nc.vector.scalar_tensor_tensor(
                ee[p][:], es[p][:], NEG, es[p][:], op0=Alu.mult, op1=Alu.max)
            nc.scalar.activation(ex[p][:], ee[p][:], Act.Exp)
            # mkP = (mk*cnt) (*) exp(e)   (cnt pre-folded on host)
            nc.vector.tensor_tensor(
                mkP[p][:].rearrange("q (t j) -> q t j", j=32),
                s["mk"][:].rearrange("q (t j) -> q t j", j=32),
                ex[p][:, :, None].broadcast_to((128, T, 32)),
                op=Alu.mult)

        def agg(p):
            s = st[p]
            yb = yps[p]
            for t in range(T):
                nc.tensor.matmul(
                    yb[32 * (t % 4):32 * (t % 4) + 32,
                       ycol(t // 4):ycol(t // 4) + SA],
                    mkP[p][:, 32 * t:32 * (t + 1)],
                    s["xab"][:, SA * t:SA * (t + 1)],
                    start=True, stop=True,
                    tile_position=(0, 32 * (t % 4)))

        def norm_proj(p):
            yb = yps[p]
            nc.vector.tensor_scalar(
                dn[p][:, 0:7],
                yb[:, 0:7 * SA].rearrange("q (b f) -> q b f", f=SA)[:, :, S],
                eps, None, op0=Alu.add)
            nc.vector.tensor_scalar(
                dn[p][:, 7:8], yb[:, 512 + S:512 + S + 1], eps, None,
                op0=Alu.add)
            nc.vector.reciprocal_approx_fast(rp[p][:], dn[p][:])
            nc.vector.tensor_tensor(
                ynrm[p][:].rearrange("q (b f) -> q b f", f=S)[:, 0:7],
                yb[:, 0:7 * SA].rearrange("q (b f) -> q b f", f=SA)[:, :, 0:S],
                rp[p][:, 0:7, None].broadcast_to((128, 7, S)),
                op=Alu.mult)
            nc.vector.tensor_tensor(
                ynrm[p][:, 7 * S:8 * S],
                yb[:, 512:512 + S],
                rp[p][:, 7:8].broadcast_to((128, S)),
                op=Alu.mult)
            for b in range(NB):
                nc.tensor.transpose(
                    ytp[:, 128 * b:128 * (b + 1)],
                    ynrm[p][:, S * b:S * (b + 1)],
                    identb, tile_position=(0, 0))
            nc.vector.tensor_copy(ynT[p][:], ytp[:])
            for i in range(2):
                nc.tensor.matmul(hT[:, 512 * i:512 * (i + 1)], Ws[p],
                                 ynT[p][:, 512 * i:512 * (i + 1)],
                                 start=True, stop=True)
            nc.scalar.activation(sg[p][:], hT[:], Act.Tanh,
                                 bias=biases[p], scale=0.5)

        front("u", 0)
        front("d", 1)
        agg("u")
        agg("d")
        norm_proj("u")
        norm_proj("d")

        nc.vector.tensor_tensor(prod[:], sg["u"][:], sg["d"][:], op=Alu.mult)
        for i in range(2):
            for k, rhs in enumerate((prod, sg["u"], sg["d"])):
                nc.tensor.matmul(hT[0:1, 512 * i:512 * (i + 1)], mlpW,
                                 rhs[:, 512 * i:512 * (i + 1)],
                                 start=(k == 0), stop=(k == 2))
            nc.vector.tensor_scalar(
                o_sb[:, 512 * i:512 * (i + 1)],
                hT[0:1, 512 * i:512 * (i + 1)], mlpb, None, op0=Alu.add)
        nc.sync.dma_start(out_dram.ap(), o_sb[:])

    nc.compile()
    return nc


def _get_module():
    if "nc" not in _CACHE:
        _CACHE["nc"] = _build_module()
    return _CACHE["nc"]


def _nz_counts(ei):
    src = np.asarray(ei[0]).astype(np.int64)
    dst = np.asarray(ei[1]).astype(np.int64)
    valid = (dst % NPG) == (NPG - 1)
    return np.bincount(src[valid], minlength=N).astype(np.float32)


def _pack_core(cnt_u, cnt_d, base):
    """First-fit-decreasing whole-graph packing for one core (shared
    graph->tile map for both branches)."""
    ku = [np.nonzero(cnt_u[base + NPG * g: base + NPG * (g + 1)])[0]
          for g in range(GPC)]
    kd = [np.nonzero(cnt_d[base + NPG * g: base + NPG * (g + 1)])[0]
          for g in range(GPC)]
    nu = np.array([len(k) for k in ku])
    nd = np.array([len(k) for k in kd])
    order = np.argsort(-(nu + nd), kind="stable")
    bins = []
    for g in order:
        placed = False
        for b in bins:
            if b[0] + nu[g] <= 128 and b[1] + nd[g] <= 128 and len(b[2]) < 32:
                b[0] += nu[g]
                b[1] += nd[g]
                b[2].append(g)
                placed = True
                break
        if not placed:
            bins.append([nu[g], nd[g], [g]])
    assert len(bins) <= T, f"packing needs {len(bins)} tiles > {T}"
    return [b[2] for b in bins], ku, kd


def _branch_arrays(tiles, klists, cnt, x, base):
    import ml_dtypes
    bf = ml_dtypes.bfloat16
    f8 = ml_dtypes.float8_e4m3
    SLOTS = T * 128
    xs = np.zeros((SLOTS, S), np.float32)
    cs = np.zeros(SLOTS, np.float32)
    mk = np.zeros((T, 128, 32), np.float32)
    mkT = np.zeros((128, NB * 128), np.float32)
    for t, gs in enumerate(tiles):
        off = 0
        for j, g in enumerate(gs):
            nodes = base + NPG * g + klists[g]
            k = len(nodes)
            xs[128 * t + off:128 * t + off + k] = x[nodes]
            cs[128 * t + off:128 * t + off + k] = cnt[nodes]
            mk[t, off:off + k, j] = cnt[nodes]
            mkT[32 * (t % 4) + j, 128 * (t // 4) + off:
                128 * (t // 4) + off + k] = 1.0
            off += k

    xab = np.zeros((T, 128, SA), np.float32)
    xab[:, :, :S] = xs.reshape(T, 128, S)
    xab[:, :, S] = 1.0
    xab = np.ascontiguousarray(
        xab.transpose(1, 0, 2).reshape(128, T * SA)).astype(bf)
    # xt: 2-block chunks — chunk c = even tile 2c (rows 0:64) and odd
    # tile 2c+1 (rows 64:128), feat-major
    xst = xs.reshape(T, 128, S)
    xtv = np.concatenate([
        xst[0::2].reshape(NCH * 128, S).T,
        xst[1::2].reshape(NCH * 128, S).T], axis=0)
    xtv = np.ascontiguousarray(xtv).astype(bf)
    cnt_t = np.ascontiguousarray(cs.reshape(T, 128).T)
    mk2 = np.ascontiguousarray(
        mk.transpose(1, 0, 2).reshape(128, T * 32)).astype(f8)
    return {"xab": xab, "xt": xtv, "mk": mk2, "mkT": mkT.astype(f8),
            "cnt": cnt_t}


def _build_in_maps(inputs):
    import ml_dtypes
    bf = ml_dtypes.bfloat16
    data = {}
    for p, pref in (("u", "up"), ("d", "down")):
        W = np.asarray(inputs[f"{pref}_W"], np.float32)
        data[p] = {
            "x": np.asarray(inputs[f"{pref}_x"], np.float32),
            "cnt": _nz_counts(inputs[f"{pref}_edge_index"]),
            "ws": W @ np.asarray(inputs[f"{pref}_att_src"], np.float32),
            "wd": W @ np.asarray(inputs[f"{pref}_att_dst"], np.float32),
            "W": W,
            "bias": np.asarray(inputs[f"{pref}_bias"], np.float32).reshape(H),
        }

    cbf = np.zeros((128, CBFW), np.float32)
    cbf[0:S, CB_V2U] = data["u"]["ws"]
    cbf[S:, CB_V2U + 1] = data["u"]["ws"]
    cbf[0:S, CB_V2D] = data["d"]["ws"]
    cbf[S:, CB_V2D + 1] = data["d"]["ws"]
    cbf[0:S, CB_D2U] = data["u"]["wd"]
    cbf[0:S, CB_D2D] = data["d"]["wd"]
    cbf[np.arange(128), CB_BM4 + np.arange(128) // 32] = 1.0
    cbl = np.zeros((128, CBLW), np.float32)
    cbl[:, CB_ID:CB_ID + 128] = np.eye(128)
    cbl[0:S, CB_WSU:CB_WSU + 128] = data["u"]["W"]
    cbl[0:S, CB_WSD:CB_WSD + 128] = data["d"]["W"]
    mw = np.asarray(inputs["mlp_W"], np.float32).reshape(H)
    cbl[:, CB_MLP] = 0.25 * mw
    cbf = cbf.astype(bf)
    cbl = cbl.astype(bf)

    cf = np.zeros((128, CFW), np.float32)
    cf[:, CF_EPS] = 1e-16
    mw = np.asarray(inputs["mlp_W"], np.float32).reshape(H)
    cf[0, CF_MLB] = float(np.asarray(inputs["mlp_b"]).reshape(-1)[0]) \
        + 0.25 * float(mw.sum())
    cf[:, CF_BU] = 0.5 * data["u"]["bias"]
    cf[:, CF_BD] = 0.5 * data["d"]["bias"]

    in_maps = []
    perms = []
    for c in range(NC):
        base = c * NPC
        tiles, ku, kd = _pack_core(data["u"]["cnt"], data["d"]["cnt"], base)
        gd_of_g = np.full(GPC, -1, np.int64)
        for t, gs in enumerate(tiles):
            for j, g in enumerate(gs):
                gd_of_g[g] = 32 * t + j
        assert (gd_of_g >= 0).all()
        perms.append(gd_of_g)
        m = {"cbL": cbl}
        cfc = cf.copy()
        for p, kl in (("u", ku), ("d", kd)):
            arrs = _branch_arrays(tiles, kl, data[p]["cnt"],
                                  data[p]["x"], base)
            for kk in ("xab", "xt", "mk", "mkT"):
                m[f"{p}_{kk}"] = arrs[kk]
            if p == "u":
                m["u_xt"] = np.concatenate([m["u_xt"], cbf], axis=1)
            # x_last feat-major by gd slot
            xlT = np.zeros((S, GD), np.float32)
            gl = np.arange(GPC)
            last = data[p]["x"][base + NPG * gl + NPG - 1]   # [GPC, S]
            xlT[:, gd_of_g] = last.T
            m[f"{p}_xlT"] = xlT.astype(bf)
        m["cf"] = cfc
        in_maps.append(m)
    return in_maps, perms


def kernel(**inputs):
    from concourse.bass_utils import run_bass_kernel_spmd

    nc = _get_module()
    in_maps, perms = _build_in_maps(inputs)
    res = run_bass_kernel_spmd(nc, in_maps, core_ids=list(range(NC)))
    out = np.empty((NC, GPC), np.float32)
    for c, r in enumerate(res.results):
        full = np.asarray(r["out"], np.float32).reshape(GD)
        out[c] = full[perms[c]]
    return out.reshape(G, 1)
